# revision 1
# baseline (speedup 1.0000x reference)
"""GAT (2-layer graph attention network) Trainium2 Bass kernel, 8-core SPMD.

Sharding: core c computes head c of layer 1 (head-parallel) and rows
[c*512,(c+1)*512) of the single-head output layer (row-parallel), with a
ReduceScatter+AllGather exchange of the per-head h @ Wo partial products.

Key identity: exp(leaky_relu(s, a)) = max(exp(s), exp(a*s)) for 0<a<=1, and
s = f_src[i] + f_dst[j] makes each exp an outer product. The per-j factor
exp(f_dst[j]) is folded into the matmul weights (including the appended
ones-column that produces the softmax denominator), so the N^2 inner loop is
one ACT scale pass, one DVE max, one DVE mask-multiply, and the PE matmul.
Constant shifts C_SRC/C_DST keep fp16 intermediates in range; they cancel in
the softmax normalization exactly.

kernel(**inputs) takes full unsharded inputs, returns the full output.
"""

from contextlib import ExitStack

import numpy as np

import concourse.mybir as mybir
import concourse.tile as tile
from concourse import bacc
from concourse.bass_utils import run_bass_kernel_spmd
from concourse.masks import make_identity

# Steer every activation to the one ACT table set covering all functions this
# kernel uses (Exp, Identity, Ln) so no mid-kernel table reloads are needed.
# Set IDs are positions in act_info.json's list, so keep the dict order and
# blank out the other sets rather than filtering them.
_orig_get_tables = bacc.get_activation_tables


def _pinned_tables(arch):
    tabs = _orig_get_tables(arch)
    if "natural_log_exp_and_others" in tabs:
        return {name: (funcs if name == "natural_log_exp_and_others" else set())
                for name, funcs in tabs.items()}
    return tabs


bacc.get_activation_tables = _pinned_tables

N = 4096
F = 512
D = 64          # per-head hidden == n classes
H = 8
ALPHA = 0.2
N_CORES = 8
P = 128
NT = N // P             # 32 node tiles
SLICE = N // N_CORES    # 512 rows per core in layer 2
NKF = F // P            # 4 contraction tiles for x @ W
C_SRC = 2.0             # constant exp shifts (cancel in softmax)
C_DST = 1.0
SPLIT_A = 2048          # layer-1 attention piece-A columns (piece B = N - SPLIT_A)
RS_A = SPLIT_A // N_CORES
RS_B = (N - SPLIT_A) // N_CORES

F32 = mybir.dt.float32
F16 = mybir.dt.float16

_CACHED = {}


def _weight_prep(nc, pools, tag, dt_fast, src_fn, extra_fn=None,
                 t_order=None):
    """Per node tile t, src_fn(t) emits + returns a [P, D+2] staging AP
    (Wh cols | f_src | f_dst). Builds scaled lhsT tiles
    whs = exp(f_dst - C_DST) * [Wh | 1] and r = exp(-(1-ALPHA) f_dst)."""
    persist, small = pools["persist"], pools["small"]
    whs_tiles = {}
    r_sb = persist.tile([P, NT], F32, tag=f"r_{tag}", name=f"r_{tag}")
    for t in (t_order if t_order is not None else range(NT)):
        src = src_fn(t)
        whs = persist.tile([P, D + 1], dt_fast, tag=f"whs_{tag}_{t}",
                           name=f"whs_{tag}_{t}")
        e1 = small.tile([P, 1], F32, tag="e1", bufs=4, name=f"e1_{tag}_{t}")
        nc.scalar.activation(e1[:], src[:, D + 1:D + 2],
                             mybir.ActivationFunctionType.Exp,
                             bias=_CACHED["bias_d"][:])
        nc.scalar.activation(r_sb[:, t:t + 1], src[:, D + 1:D + 2],
                             mybir.ActivationFunctionType.Exp,
                             scale=-(1.0 - ALPHA))
        nc.vector.tensor_scalar_mul(whs[:, 0:D], src[:, 0:D], e1[:])
        nc.vector.tensor_copy(whs[:, D:D + 1], e1[:])
        if extra_fn is not None:
            extra_fn(t, src)
        whs_tiles[t] = whs
    return whs_tiles, r_sb


def _bcast_exp_rows(nc, pools, tag, dt_fast, row_ap, width):
    """From row_ap [1, width] (raw f_src on partition 0), build [P, width]
    tiles F1 = exp(f_src - C_SRC), F2 = exp(ALPHA*f_src - C_SRC)."""
    persist, psum = pools["persist"], pools["psum"]
    ones1 = _CACHED["ones16"] if row_ap.dtype == F16 else _CACHED["ones1"]
    f1 = persist.tile([P, width], dt_fast, tag=f"f1_{tag}", name=f"f1_{tag}")
    f2 = persist.tile([P, width], dt_fast, tag=f"f2_{tag}", name=f"f2_{tag}")
    for c in range(width // 512):
        bc_ps = psum.tile([P, 512], F32, tag="bank", bufs=8,
                          name=f"bc_{tag}_{c}")
        nc.tensor.matmul(bc_ps[:], ones1[0:1, :],
                         row_ap[0:1, c * 512:(c + 1) * 512],
                         start=True, stop=True)
        nc.scalar.activation(f1[:, c * 512:(c + 1) * 512], bc_ps[:],
                             mybir.ActivationFunctionType.Exp,
                             bias=_CACHED["bias_s"][:])
        nc.scalar.activation(f2[:, c * 512:(c + 1) * 512], bc_ps[:],
                             mybir.ActivationFunctionType.Exp,
                             bias=_CACHED["bias_s"][:], scale=ALPHA)
    return f1, f2


CCE_MUL = False


def _attention(nc, pools, tag, dt_fast, whs_tiles, r_sb, f1_bc, f2_bc,
               adjt_ap, out_sb, i_width, ew_widths, j_order=None,
               half_cb=None):
    """Masked-softmax attention: out_sb[d, i] = sum_j attn[i,j] Wh[j,d].
    adjt_ap: DRAM AP [N, i_width] (adj^T slice, fp16). After each ew_width
    i-half completes (including normalization), half_cb(ih) is invoked so the
    caller can pipeline downstream work (ELU, h@Wo, collectives) against the
    next half's attention loop."""
    if j_order is None:
        j_order = list(range(NT))
    if isinstance(ew_widths, int):
        ew_widths = [ew_widths]
    assert sum(ew_widths) == i_width
    psum, work, small, dram = (pools["psum"], pools["work"], pools["small"],
                               pools["dram"])
    ones1 = _CACHED["ones1"]

    i0 = 0
    for ih, ew_width in enumerate(ew_widths):
        n_chunk = ew_width // 512
        accs = [psum.tile([D + 1, 512], F32, tag="bank", bufs=8,
                          name=f"acc_{tag}_{ih}_{q}") for q in range(n_chunk)]
        for jn, j in enumerate(j_order):
            u_t = work.tile([P, ew_width], dt_fast, tag="u", bufs=8,
                            name=f"u_{tag}_{ih}_{jn}")
            nc.scalar.activation(u_t[:], f2_bc[:, i0:i0 + ew_width],
                                 mybir.ActivationFunctionType.Identity,
                                 scale=r_sb[:, j:j + 1])
            nc.vector.tensor_max(u_t[:], u_t[:], f1_bc[:, i0:i0 + ew_width])
            adj_t = work.tile([P, ew_width], F16, tag="adj", bufs=10,
                              name=f"adj_{tag}_{ih}_{jn}")
            nc.sync.dma_start(out=adj_t[:],
                              in_=adjt_ap[j * P:(j + 1) * P,
                                          i0:i0 + ew_width])
            nc.vector.tensor_mul(adj_t[:], u_t[:], adj_t[:])
            for q in range(n_chunk):
                nc.tensor.matmul(accs[q][:], whs_tiles[j][:],
                                 adj_t[:, q * 512:(q + 1) * 512],
                                 start=(jn == 0), stop=(jn == NT - 1))

        # ---- per-half normalization tail ----
        # Broadcast the PSUM denominator row (partition D) down D partitions
        # with a base-64 ones-matmul, then reciprocal as exp(-ln(x)) on ACT
        # (Ln/Exp share one table set) — no cross-partition DMAs needed.
        ones64t = _CACHED["ones64t"]
        for q in range(n_chunk):
            num_sb = work.tile([D + 1, 512], F32, tag="num", bufs=4,
                               name=f"num_{tag}_{ih}_{q}")
            nc.scalar.activation(num_sb[:], accs[q][:],
                                 mybir.ActivationFunctionType.Copy)
            den_bc = psum.tile([D, 512], F32, tag="bank", bufs=8,
                               name=f"denbc_{tag}_{ih}_{q}")
            nc.tensor.matmul(den_bc[:], ones64t[D:D + 1, :],
                             num_sb[D:D + 1, :], start=True, stop=True)
            lnb = work.tile([D, 512], F32, tag="lnb", bufs=2,
                            name=f"lnb_{tag}_{ih}_{q}")
            nc.scalar.activation(lnb[:], den_bc[:],
                                 mybir.ActivationFunctionType.Ln)
            recb = work.tile([D, 512], F32, tag="recb", bufs=2,
                             name=f"recb_{tag}_{ih}_{q}")
            nc.scalar.activation(recb[:], lnb[:],
                                 mybir.ActivationFunctionType.Exp, scale=-1.0)
            nc.vector.tensor_mul(out_sb[:, i0 + q * 512:i0 + (q + 1) * 512],
                                 num_sb[0:D, :], recb[:])
        if half_cb is not None:
            half_cb(ih)
        i0 += ew_width


def _elu(nc, pools, tag, src_ap, dst_ap, width, ew=512):
    """dst = elu(src) elementwise on [D, width] fp32 tiles."""
    work = pools["work"]
    for s in range(width // ew):
        sl = slice(s * ew, (s + 1) * ew)
        t_min = work.tile([D, ew], F32, tag="elu_min", bufs=2,
                          name=f"elmin_{tag}_{s}")
        nc.vector.tensor_scalar_min(t_min[:], src_ap[:, sl], 0.0)
        t_exp = work.tile([D, ew], F32, tag="elu_exp", bufs=2,
                          name=f"elexp_{tag}_{s}")
        nc.scalar.activation(t_exp[:], t_min[:],
                             mybir.ActivationFunctionType.Exp)
        t_lin = work.tile([D, ew], F32, tag="elu_lin", bufs=2,
                          name=f"ellin_{tag}_{s}")
        nc.vector.tensor_scalar(t_lin[:], src_ap[:, sl], 0.0, -1.0,
                                mybir.AluOpType.max, mybir.AluOpType.add)
        nc.vector.tensor_add(dst_ap[:, sl], t_exp[:], t_lin[:])


def build_kernel(dt_fast=F16, repeat=1):
    nc = bacc.Bacc("TRN2", num_devices=N_CORES)

    xT = nc.dram_tensor("xT", [F, N], F16, kind="ExternalInput")
    adjT = nc.dram_tensor("adjT", [N, N], F16, kind="ExternalInput")
    adjT2 = nc.dram_tensor("adjT2", [N, SLICE], F16, kind="ExternalInput")
    Wext = nc.dram_tensor("Wext", [F, D + 2], F16, kind="ExternalInput")
    Woext = nc.dram_tensor("Woext", [D, D + 2], F32, kind="ExternalInput")
    outT = nc.dram_tensor("outT", [D, SLICE], F32, kind="ExternalOutput")

    with ExitStack() as ctx:
        tc = ctx.enter_context(tile.TileContext(nc))
        psum = ctx.enter_context(tc.tile_pool(name="psum", bufs=1, space="PSUM"))
        persist = ctx.enter_context(tc.tile_pool(name="persist", bufs=1))
        work = ctx.enter_context(tc.tile_pool(name="work", bufs=1))
        small = ctx.enter_context(tc.tile_pool(name="small", bufs=1))
        dram = ctx.enter_context(tc.tile_pool(name="dram", bufs=1, space="DRAM"))
        pools = {"psum": psum, "persist": persist, "work": work,
                 "small": small, "dram": dram}

        ident = persist.tile([P, P], F32, tag="ident")
        make_identity(nc, ident[:])
        ones1 = persist.tile([1, P], F32, tag="ones1")
        nc.vector.memset(ones1[:], 1.0)
        ones16 = persist.tile([1, P], F16, tag="ones16")
        nc.vector.memset(ones16[:], 1.0)
        _CACHED.clear()
        _CACHED["ones1"] = ones1
        _CACHED["ident"] = ident
        _CACHED["ones16"] = ones16
        bias_s = persist.tile([P, 1], F32, tag="bias_s")
        nc.vector.memset(bias_s[:], -C_SRC)
        bias_d = persist.tile([P, 1], F32, tag="bias_d")
        nc.vector.memset(bias_d[:], -C_DST)
        _CACHED["bias_s"] = bias_s
        _CACHED["bias_d"] = bias_d
        ones64t = persist.tile([D + 1, D], F32, tag="ones64t")
        nc.vector.memset(ones64t[:], 1.0)
        _CACHED["ones64t"] = ones64t

        def emit_body():
            _emit_gat(nc, pools, dt_fast, xT, adjT, adjT2, Wext, Woext, outT)

        for _rep in range(repeat):
            emit_body()

    nc.compile()
    return nc


def _emit_gat(nc, pools, dt_fast, xT, adjT, adjT2, Wext, Woext, outT):
    psum, persist, work, small, dram = (pools["psum"], pools["persist"],
                                        pools["work"], pools["small"],
                                        pools["dram"])
    if True:
        # ---- phase 1: Wh = x @ W_ext (fp16 in, fp32 accum), weight prep ----
        wext_sb = []
        for kf in range(NKF):
            t = small.tile([P, D + 2], F16, tag=f"wext{kf}", name=f"wext_{kf}")
            nc.sync.dma_start(out=t[:], in_=Wext[kf * P:(kf + 1) * P, :])
            wext_sb.append(t)
        xt_sb = []
        for kf in range(NKF):
            t = work.tile([P, N], F16, tag=f"xt{kf}", name=f"xt_{kf}")
            nc.sync.dma_start(out=t[:], in_=xT[kf * P:(kf + 1) * P, :])
            xt_sb.append(t)

        def l1_src(t):
            wh_ps = psum.tile([P, D + 2], F32, tag="bank", bufs=8,
                              name=f"whps_{t}")
            for kf in range(NKF):
                nc.tensor.matmul(wh_ps[:], xt_sb[kf][:, t * P:(t + 1) * P],
                                 wext_sb[kf][:], start=(kf == 0),
                                 stop=(kf == NKF - 1))
            return wh_ps

        whs1, r1_sb = _weight_prep(nc, pools, "l1", dt_fast, l1_src)

        # f_src row directly: fs_row[i] = sum_f wa_src[f] * xT[f,i] via M=1
        # matmuls (fp16, same precision as the Wh path), no transpose/bounce
        fs_row = work.tile([1, N], F32, tag="bigrow", name="fs_row")
        for sl in range(8):
            fr_ps = psum.tile([1, 512], F32, tag="bank", bufs=8,
                              name=f"frps_{sl}")
            for kf in range(NKF):
                nc.tensor.matmul(fr_ps[:], wext_sb[kf][:, D:D + 1],
                                 xt_sb[kf][:, sl * 512:(sl + 1) * 512],
                                 start=(kf == 0), stop=(kf == NKF - 1))
            nc.scalar.activation(fs_row[0:1, sl * 512:(sl + 1) * 512],
                                 fr_ps[:], mybir.ActivationFunctionType.Copy)
        f1_bc, f2_bc = _bcast_exp_rows(nc, pools, "l1", dt_fast, fs_row, N)

        # ---- phases 2+3 pipelined: layer-1 attention in two uneven
        # i-pieces (3072 + 1024); after each piece: ELU, h @ Wo_ext and its
        # own ReduceScatter+AllGather, so piece A's exchange and most of the
        # serial tail hide under piece B's attention loop. Core c's layer-2
        # rows are [c*384,(c+1)*384) of piece A plus [c*128,(c+1)*128) of B.
        HALF_COLS = [SPLIT_A, N - SPLIT_A]
        HALF_TILES = [SPLIT_A // P, (N - SPLIT_A) // P]
        RS_ROWS = [hc // N_CORES for hc in HALF_COLS]      # 384, 128
        o1_sb = persist.tile([D, N], F32, tag="o1")
        woext_sb = persist.tile([D, D + 2], F32, tag="woext")
        nc.sync.dma_start(out=woext_sb[:], in_=Woext[:])
        cc_in = [dram.tile([HALF_COLS[h], D + 2], F16, tag=f"cc_in{h}",
                           name=f"cc_in{h}") for h in range(2)]
        cc_rs = [dram.tile([RS_ROWS[h], D + 2], F16, tag=f"cc_rs{h}",
                           name=f"cc_rs{h}") for h in range(2)]
        cc_full = [dram.tile([HALF_COLS[h], D + 2], F16, tag=f"cc_full{h}",
                             addr_space="Shared", name=f"cc_full{h}")
                   for h in range(2)]

        def l1_half_done(h):
            lo = sum(HALF_COLS[:h])
            t0 = sum(HALF_TILES[:h])
            _elu(nc, pools, f"l1h{h}", o1_sb[:, lo:lo + HALF_COLS[h]],
                 o1_sb[:, lo:lo + HALF_COLS[h]], HALF_COLS[h])
            for tt in range(HALF_TILES[h]):
                t = t0 + tt
                p2_ps = psum.tile([P, D + 2], F32, tag="bank", bufs=8,
                                  name=f"p2ps_{t}")
                nc.tensor.matmul(p2_ps[:], o1_sb[:, t * P:(t + 1) * P],
                                 woext_sb[:], start=True, stop=True)
                p2_sb = work.tile([P, D + 2], F16, tag="stage66", bufs=4,
                                  name=f"p2sb_{t}")
                nc.scalar.activation(p2_sb[:], p2_ps[:],
                                     mybir.ActivationFunctionType.Copy)
                nc.sync.dma_start(out=cc_in[h][tt * P:(tt + 1) * P, :],
                                  in_=p2_sb[:])
            nc.gpsimd.collective_compute(
                "ReduceScatter", mybir.AluOpType.add,
                ins=[cc_in[h][:]], outs=[cc_rs[h][:]],
                replica_groups=[list(range(N_CORES))])
            nc.gpsimd.collective_compute(
                "AllGather", mybir.AluOpType.bypass,
                ins=[cc_rs[h][:]], outs=[cc_full[h][:]],
                replica_groups=[list(range(N_CORES))])

        _attention(nc, pools, "l1", dt_fast, whs1, r1_sb, f1_bc, f2_bc,
                   adjT[:], o1_sb, N, HALF_COLS, half_cb=l1_half_done)

        # ---- phase 4: layer-2 prep (piece-A tiles arrive first) ----
        def l2_src(t):
            h = 0 if t < HALF_TILES[0] else 1
            tt = t - (0 if h == 0 else HALF_TILES[0])
            s = work.tile([P, D + 2], F16, tag="ccsb", bufs=4,
                          name=f"ccsb_{t}")
            nc.sync.dma_start(out=s[:],
                              in_=cc_full[h][tt * P:(tt + 1) * P, :])
            return s

        whs2, r2_sb = _weight_prep(nc, pools, "l2", dt_fast, l2_src)

        fs2_row = small.tile([1, SLICE], F16, tag="fs2_row")
        off = 0
        for h in range(2):
            nc.sync.dma_start(
                out=fs2_row[0:1, off:off + RS_ROWS[h]],
                in_=cc_rs[h][:, D:D + 1].rearrange("n one -> one n"))
            off += RS_ROWS[h]
        f1_bc2, f2_bc2 = _bcast_exp_rows(nc, pools, "l2", dt_fast, fs2_row,
                                         SLICE)

        # ---- layer-2 attention on this core's row slices, ELU, store ----
        o2_sb = persist.tile([D, SLICE], F32, tag="o2")
        _attention(nc, pools, "l2", dt_fast, whs2, r2_sb, f1_bc2, f2_bc2,
                   adjT2[:], o2_sb, SLICE, [512])
        fin = persist.tile([D, SLICE], F32, tag="fin")
        _elu(nc, pools, "l2", o2_sb, fin, SLICE)
        nc.sync.dma_start(out=outT[:], in_=fin[:])


# ---------------------------------------------------------------------------
# host-side driver
# ---------------------------------------------------------------------------

def _prep_inputs(x, adj, W, a, Wo, ao):
    xT = np.ascontiguousarray(x.T.astype(np.float16))
    adjT = np.ascontiguousarray(adj.T.astype(np.float16))
    in_maps = []
    for c in range(N_CORES):
        a_src, a_dst = a[c, :D], a[c, D:]
        wext = np.concatenate(
            [W[c], (W[c] @ a_src)[:, None], (W[c] @ a_dst)[:, None]],
            axis=1).astype(np.float16)
        Wo_h = Wo[c * D:(c + 1) * D]
        woext = np.concatenate(
            [Wo_h, (Wo_h @ ao[:D])[:, None], (Wo_h @ ao[D:])[:, None]],
            axis=1).astype(np.float32)
        adjt2 = np.concatenate(
            [adjT[:, c * RS_A:(c + 1) * RS_A],
             adjT[:, SPLIT_A + c * RS_B:SPLIT_A + (c + 1) * RS_B]], axis=1)
        in_maps.append({
            "xT": xT,
            "adjT": adjT,
            "adjT2": np.ascontiguousarray(adjt2),
            "Wext": wext,
            "Woext": woext,
        })
    return in_maps


def kernel(x, adj, W, a, Wo, ao, cfg):
    x = np.asarray(x, np.float32)
    adj = np.asarray(adj, np.float32)
    W = np.asarray(W, np.float32)
    a = np.asarray(a, np.float32)
    Wo = np.asarray(Wo, np.float32)
    ao = np.asarray(ao, np.float32)

    in_maps = _prep_inputs(x, adj, W, a, Wo, ao)
    if _CACHED.get("nc") is None:
        nc = build_kernel()
        _CACHED["nc"] = nc
    res = run_bass_kernel_spmd(_CACHED["nc"], in_maps,
                               core_ids=list(range(N_CORES)))
    out = np.empty((N, D), np.float32)
    for c in range(N_CORES):
        oT = res.results[c]["outT"]
        out[c * RS_A:(c + 1) * RS_A, :] = oT[:, 0:RS_A].T
        out[SPLIT_A + c * RS_B:SPLIT_A + (c + 1) * RS_B, :] = oT[:, RS_A:].T
    return out


if __name__ == "__main__":
    import reference as ref_mod
    inputs = {k: np.asarray(v) for k, v in ref_mod.setup_inputs().items()}
    expected = np.asarray(ref_mod.reference(**ref_mod.setup_inputs()))
    got = kernel(**inputs)
    err = np.abs(got - expected).max() / np.abs(expected).max()
    print("rel err:", err)



# revision 11
# speedup vs baseline: 1.1655x; 1.1655x over previous
"""GAT (2-layer graph attention network) Trainium2 Bass kernel, 8-core SPMD.

Sharding (v2): every core computes ALL 8 layer-1 heads but only its own
512-column i-slice of the attention output (column-parallel), and the same
i-slice of layer 2. The adjacency slice [4096, 512] is loaded once per core
(two big DMAs) and reused by all 8 heads AND layer 2. Layer-1 -> layer-2
exchange is an AllGather of the per-core h @ Wo rows, split into a 384-row
and a 128-row piece so the first gather hides under the second piece's
attention loop.

Key math: exp(leaky_relu(s)) with s = f_src_i + f_dst_j factorizes as
e^{f_i} * max(g_i * r'_j, e1_j) with g_i = e^{(a-1) f_i},
r'_j = e^{a f_dst_j - C}, e1_j = e^{f_dst_j - C}. The e^{f_i} factor cancels
in the softmax, so the inner loop per (head, j-tile) is ONE
tensor_scalar (mult+max with two per-partition scalars, 4x DVE mode) and one
mask multiply (tensor_tensor, batched over 4 j-tiles), feeding a PE matmul
whose lhsT is the raw [Wh | 1] tile (ones column accumulates the softmax
denominator).

kernel(**inputs) takes full unsharded inputs, returns the full output.
"""

from contextlib import ExitStack

import numpy as np

import concourse.mybir as mybir
import concourse.tile as tile
from concourse import bacc
from concourse.bass_utils import run_bass_kernel_spmd
from concourse.masks import make_identity

# Steer every activation to the one ACT table set covering all functions this
# kernel uses (Exp, Copy, Identity) so no mid-kernel table reloads happen.
_orig_get_tables = bacc.get_activation_tables


def _pinned_tables(arch):
    tabs = _orig_get_tables(arch)
    if "natural_log_exp_and_others" in tabs:
        return {name: (funcs if name == "natural_log_exp_and_others" else set())
                for name, funcs in tabs.items()}
    return tabs


bacc.get_activation_tables = _pinned_tables

N = 4096
F = 512
D = 64          # per-head hidden == n classes
H = 8
P = 128
NT = N // P             # 32 j tiles
NKF = F // P            # 4 contraction tiles for x @ W
SLICE = N // 8          # 512 i columns per core
ALPHA = 0.2
AM1 = ALPHA - 1.0       # -0.8
C_DST = 1.0
PIECES = [(0, 384), (384, 128)]   # i-piece (offset, width) within the slice
N_CORES = 8
E = D + 2               # 66: [Wh | f_src | f_dst]

F32 = mybir.dt.float32
F16 = mybir.dt.float16

_CACHED = {}

AF = mybir.ActivationFunctionType
ALU = mybir.AluOpType


def build_kernel():
    nc = bacc.Bacc("TRN2", num_devices=N_CORES)

    xtr = nc.dram_tensor("xtr", [P, NKF * N], F16, kind="ExternalInput")
    xslr = nc.dram_tensor("xslr", [P, NKF * SLICE], F16, kind="ExternalInput")
    adjc = nc.dram_tensor("adjc", [P, NT * SLICE], F16, kind="ExternalInput")
    wextr = nc.dram_tensor("wextr", [P, NKF * H * E], F16,
                           kind="ExternalInput")
    woA = nc.dram_tensor("woA", [D, H * E], F16, kind="ExternalInput")
    outT = nc.dram_tensor("outT", [D, SLICE], F32, kind="ExternalOutput")

    with ExitStack() as ctx:
        tc = ctx.enter_context(tile.TileContext(nc))
        psum = ctx.enter_context(tc.tile_pool(name="psum", bufs=1, space="PSUM"))
        persist = ctx.enter_context(tc.tile_pool(name="persist", bufs=1))
        work = ctx.enter_context(tc.tile_pool(name="work", bufs=1))
        dram = ctx.enter_context(tc.tile_pool(name="dram", bufs=1, space="DRAM"))
        pools = {"psum": psum, "persist": persist, "work": work, "dram": dram}

        ident = persist.tile([P, P], F32, tag="ident")
        make_identity(nc, ident[:])
        ones_all = persist.tile([P, P], F32, tag="ones_all")
        nc.vector.memset(ones_all[:], 1.0)
        ones65 = persist.tile([D + 1, D], F32, tag="ones65")
        nc.vector.memset(ones65[:], 1.0)
        bias_d = persist.tile([P, 1], F32, tag="bias_d")
        nc.vector.memset(bias_d[:], -C_DST)
        _CACHED["bias_d"] = bias_d
        _CACHED["ident"] = ident
        _CACHED["ones_all"] = ones_all
        _CACHED["ones65"] = ones65

        _emit(nc, pools, xtr, xslr, adjc, wextr, woA, outT)

    nc.compile()
    return nc


def _emit(nc, pools, xtr, xslr, adjc, wextr, woA, outT):
    psum, persist, work, dram = (pools["psum"], pools["persist"],
                                 pools["work"], pools["dram"])
    ident = _CACHED["ident"]
    ones_all = _CACHED["ones_all"]
    ones65 = _CACHED["ones65"]

    # ---- input DMAs (few, large) ----
    wextr_sb = persist.tile([P, NKF * H * E], F16, tag="wextr")
    nc.sync.dma_start(out=wextr_sb[:], in_=wextr[:])
    xt_sb = persist.tile([P, NKF * N], F16, tag="xt")
    nc.sync.dma_start(out=xt_sb[:], in_=xtr[:])
    xsl_sb = persist.tile([P, NKF * SLICE], F16, tag="xsl")
    nc.sync.dma_start(out=xsl_sb[:], in_=xslr[:])
    woA_sb = persist.tile([D, H * E], F16, tag="woA")
    nc.sync.dma_start(out=woA_sb[:], in_=woA[:])
    adj_sb = persist.tile([P, NT * SLICE], F16, tag="adj")
    HALF = NT * PIECES[0][1]
    nc.sync.dma_start(out=adj_sb[:, 0:HALF], in_=adjc[:, 0:HALF])
    nc.sync.dma_start(out=adj_sb[:, HALF:], in_=adjc[:, HALF:])

    # ---- per-head prep: whs = [Wh | 1] fp16, e1 = exp(f_dst - C),
    #      r' = exp(a*f_dst - C), g_row = exp((a-1) f_src) broadcast ----
    whs, e1_sb, rp_sb, g_row = [], [], [], []
    for h in range(H):
        wb = persist.tile([P, NT * (D + 1)], F16, tag=f"whs{h}",
                          name=f"whs_{h}")
        nc.vector.memset(
            wb[:].rearrange("p (t c) -> p t c", t=NT)[:, :, D:D + 1], 1.0)
        e1 = persist.tile([P, NT], F32, tag=f"e1_{h}", name=f"e1_{h}")
        rp = persist.tile([P, NT], F32, tag=f"rp_{h}", name=f"rp_{h}")
        wb3 = wb[:].rearrange("p (t c) -> p t c", t=NT)
        for grp in range(NT // 4):
            wh_ps = psum.tile([P, 4 * E], F32, tag="bank", bufs=3,
                              name=f"whps_{h}_{grp}")
            for k in range(4):
                t = grp * 4 + k
                for kf in range(NKF):
                    nc.tensor.matmul(
                        wh_ps[:, k * E:(k + 1) * E],
                        xt_sb[:, kf * N + t * P:kf * N + (t + 1) * P],
                        wextr_sb[:, (kf * H + h) * E:(kf * H + h + 1) * E],
                        start=(kf == 0), stop=(kf == NKF - 1))
            ps3 = wh_ps[:].rearrange("p (k c) -> p k c", k=4)
            nc.scalar.activation(
                e1[:, grp * 4:(grp + 1) * 4].rearrange(
                    "p (k one) -> p k one", one=1),
                ps3[:, :, D + 1:D + 2], AF.Exp, bias=_CACHED["bias_d"][:])
            nc.scalar.activation(
                rp[:, grp * 4:(grp + 1) * 4].rearrange(
                    "p (k one) -> p k one", one=1),
                ps3[:, :, D + 1:D + 2], AF.Exp, bias=_CACHED["bias_d"][:], scale=ALPHA)
            nc.scalar.activation(
                wb3[:, grp * 4:(grp + 1) * 4, 0:D], ps3[:, :, 0:D], AF.Copy)
        whs.append(wb)
        e1_sb.append(e1)
        rp_sb.append(rp)

        fs_ps = psum.tile([1, SLICE], F32, tag="bank", bufs=3,
                          name=f"fsps_{h}")
        for kf in range(NKF):
            nc.tensor.matmul(
                fs_ps[:],
                wextr_sb[:, (kf * H + h) * E + D:(kf * H + h) * E + D + 1],
                xsl_sb[:, kf * SLICE:(kf + 1) * SLICE],
                start=(kf == 0), stop=(kf == NKF - 1))
        fs_sb = work.tile([1, SLICE], F32, tag="fs_sb", bufs=2,
                          name=f"fssb_{h}")
        nc.scalar.activation(fs_sb[:], fs_ps[:], AF.Copy)
        g_ps = psum.tile([P, SLICE], F32, tag="bank", bufs=3, name=f"gps_{h}")
        for ch in range(SLICE // P):
            nc.tensor.matmul(g_ps[:, ch * P:(ch + 1) * P],
                             ones_all[0:1, 0:P],
                             fs_sb[0:1, ch * P:(ch + 1) * P],
                             start=True, stop=True)
        gr = persist.tile([P, SLICE], F16, tag=f"g_{h}", name=f"g_{h}")
        nc.scalar.activation(gr[:], g_ps[:], AF.Exp, scale=AM1)
        g_row.append(gr)

    # ---- layer-1 attention + (pipelined) norm/ELU/p2, two i-pieces ----
    p2AB_ps = psum.tile([P, 4 * E], F32, tag="p2AB", name="p2AB")
    p2A_ps = p2AB_ps[:, 0:3 * E]
    p2B_ps = p2AB_ps[:, 3 * E:4 * E]
    cc_inA = dram.tile([3 * P, E], F16, tag="cc_inA", name="cc_inA")
    cc_inB = dram.tile([P, E], F16, tag="cc_inB", name="cc_inB")
    cc_fullA = dram.tile([N_CORES * 3 * P, E], F16, tag="cc_fullA",
                         addr_space="Shared", name="cc_fullA")
    cc_fullB = dram.tile([N_CORES * P, E], F16, tag="cc_fullB",
                         addr_space="Shared", name="cc_fullB")

    ADJ_B0 = NT * PIECES[0][1]     # start col of the B block

    def adj_piece(pi, t, ntiles=1):
        pw = PIECES[pi][1]
        base = 0 if pi == 0 else ADJ_B0
        return adj_sb[:, base + t * pw:base + (t + ntiles) * pw]

    def attention(tag, pi, pw, g_ap, rp_ap, e1_ap, whs_ap, acc):
        off = PIECES[pi][0]
        for grp in range(NT // 4):
            vg = work.tile([P, 4 * pw], F16, tag="vg", bufs=3,
                           name=f"vg_{tag}_{grp}")
            for k in range(4):
                t = grp * 4 + k
                nc.vector.tensor_scalar(
                    vg[:, k * pw:(k + 1) * pw], g_ap[:, off:off + pw],
                    rp_ap[:, t:t + 1], e1_ap[:, t:t + 1], ALU.mult, ALU.max)
            wg = work.tile([P, 4 * pw], F16, tag="wg", bufs=4,
                           name=f"wg_{tag}_{grp}")
            nc.vector.tensor_tensor(wg[:], vg[:], adj_piece(pi, grp * 4, 4),
                                    ALU.mult)
            for k in range(4):
                t = grp * 4 + k
                nc.tensor.matmul(
                    acc[:], whs_ap[:, t * (D + 1):(t + 1) * (D + 1)],
                    wg[:, k * pw:(k + 1) * pw],
                    start=(t == 0), stop=(t == NT - 1))

    def elu(tag, x_ap, out_ap, pw):
        e = work.tile([D, pw], F32, tag="elu_e", bufs=2, name=f"ele_{tag}")
        nc.scalar.activation(e[:], x_ap, AF.Exp)
        t1 = work.tile([D, pw], F32, tag="elu_t", bufs=2, name=f"elt_{tag}")
        nc.vector.tensor_scalar(t1[:], e[:], 1.0, -1.0, ALU.min, ALU.add)
        nc.vector.scalar_tensor_tensor(out_ap, x_ap, 0.0, t1[:],
                                       ALU.max, ALU.add)

    def norm_elu_p2(tag, acc, pw, h, fs2_ps):
        num = work.tile([D + 1, pw], F32, tag="num", bufs=3,
                        name=f"num_{tag}")
        nc.scalar.activation(num[:], acc[:], AF.Copy)
        lnr = work.tile([1, pw], F32, tag="lnr", bufs=2, name=f"ln_{tag}")
        nc.scalar.activation(lnr[:], num[D:D + 1, :], AF.Ln)
        rec = work.tile([1, pw], F32, tag="rec", bufs=2, name=f"rc_{tag}")
        nc.scalar.activation(rec[:], lnr[:], AF.Exp, scale=-1.0)
        den = psum.tile([D, pw], F32, tag="bank", bufs=3, name=f"den_{tag}")
        nc.tensor.matmul(den[:], ones_all[0:1, 0:D], rec[0:1, :],
                         start=True, stop=True)
        x = work.tile([D, pw], F32, tag="xat", bufs=3, name=f"x_{tag}")
        nc.vector.tensor_tensor(x[:], num[0:D, :], den[:], ALU.mult)
        o_sb = work.tile([D, pw], F16, tag="osb", bufs=10, name=f"o_{tag}")
        elu(tag, x[:], o_sb[:], pw)
        # f_src2 row accumulates in its own bank; one open group there is
        # safe. The p2 chunk matmuls are emitted after the head loop so
        # each bank has at most one open accumulation group at a time.
        nc.tensor.matmul(
            fs2_ps, woA_sb[:, h * E + D:h * E + D + 1], o_sb[:],
            start=(h == 0), stop=(h == H - 1))
        return o_sb

    p2_sb = {}
    fs2_sb = work.tile([1, SLICE], F32, tag="fs2_sb", name="fs2_sb")
    fs2all_ps = psum.tile([1, SLICE], F32, tag="fs2", name="fs2")
    for pi, (off, pw) in enumerate(PIECES):
        nch = pw // P
        p2_ps = p2A_ps if pi == 0 else p2B_ps

        fs2_ps = fs2all_ps[0:1, off:off + pw]
        pending = None
        o_list = []
        for h in range(H):
            acc = psum.tile([D + 1, pw], F32, tag="acc", bufs=2,
                            name=f"acc_{pi}_{h}")
            attention(f"l1_{pi}_{h}", pi, pw, g_row[h][:], rp_sb[h][:],
                      e1_sb[h][:], whs[h][:], acc)
            if pending is not None:
                o_list.append(norm_elu_p2(
                    f"l1_{pi}_{pending[0]}", pending[1], pw, pending[0],
                    fs2_ps))
            pending = (h, acc)
        o_list.append(norm_elu_p2(f"l1_{pi}_{pending[0]}", pending[1], pw,
                                  pending[0], fs2_ps))
        for ch in range(nch):
            for h in range(H):
                nc.tensor.matmul(
                    p2_ps[:, ch * E:(ch + 1) * E],
                    o_list[h][:, ch * P:(ch + 1) * P],
                    woA_sb[:, h * E:(h + 1) * E],
                    start=(h == 0), stop=(h == H - 1))
        nc.scalar.activation(fs2_sb[0:1, off:off + pw], fs2_ps, AF.Copy)
        ps = persist.tile([P, nch * E], F16, tag=f"p2sb_{pi}",
                          name=f"p2sb_{pi}")
        nc.scalar.activation(ps[:], p2_ps, AF.Copy)
        p2_sb[pi] = ps
        cc_in = cc_inA if pi == 0 else cc_inB
        cc_full = cc_fullA if pi == 0 else cc_fullB
        nc.sync.dma_start(
            out=cc_in[:].rearrange("(c p) d -> p c d", p=P),
            in_=ps[:].rearrange("p (c d) -> p c d", c=nch))
        nc.gpsimd.collective_compute(
            "AllGather", ALU.bypass, ins=[cc_in[:]], outs=[cc_full[:]],
            replica_groups=[list(range(N_CORES))])

    # ---- g2_row from this core's own f_src2 row ----
    g2_ps = psum.tile([P, SLICE], F32, tag="bank", bufs=3, name="g2ps")
    for ch in range(SLICE // P):
        nc.tensor.matmul(g2_ps[:, ch * P:(ch + 1) * P],
                         ones_all[0:1, 0:P], fs2_sb[0:1, ch * P:(ch + 1) * P],
                         start=True, stop=True)
    g2_row = persist.tile([P, SLICE], F16, tag="g2", name="g2")
    nc.scalar.activation(g2_row[:], g2_ps[:], AF.Exp, scale=AM1)

    # ---- layer-2 prep from gathered rows ----
    whs2 = persist.tile([P, NT * (D + 1)], F16, tag="whs2", name="whs2")
    nc.vector.memset(
        whs2[:].rearrange("p (t c) -> p t c", t=NT)[:, :, D:D + 1], 1.0)
    e1_2 = persist.tile([P, NT], F32, tag="e1_2", name="e1_2")
    rp_2 = persist.tile([P, NT], F32, tag="rp_2", name="rp_2")

    ccA_sb = persist.tile([P, 24 * E], F16, tag="ccA", name="ccA")
    nc.sync.dma_start(
        out=ccA_sb[:].rearrange("p (u d) -> p u d", u=24),
        in_=cc_fullA[:].rearrange("(u p) d -> p u d", p=P))
    ccB_sb = persist.tile([P, 8 * E], F16, tag="ccB", name="ccB")
    nc.sync.dma_start(
        out=ccB_sb[:].rearrange("p (u d) -> p u d", u=8),
        in_=cc_fullB[:].rearrange("(u p) d -> p u d", p=P))

    w23 = whs2[:].rearrange("p (t c) -> p t c", t=NT)

    def l2_prep(cc_sb, n_u, g_of_u):
        cc3 = cc_sb[:].rearrange("p (u c) -> p u c", u=n_u)
        for u in range(n_u):
            g = g_of_u(u)
            nc.scalar.activation(e1_2[:, g:g + 1], cc3[:, u, D + 1:D + 2],
                                 AF.Exp, bias=_CACHED["bias_d"][:])
            nc.scalar.activation(rp_2[:, g:g + 1], cc3[:, u, D + 1:D + 2],
                                 AF.Exp, bias=_CACHED["bias_d"][:], scale=ALPHA)
            nc.scalar.activation(w23[:, g, 0:D], cc3[:, u, 0:D], AF.Copy)

    l2_prep(ccA_sb, 24, lambda u: (u // 3) * 4 + u % 3)
    l2_prep(ccB_sb, 8, lambda u: u * 4 + 3)

    # ---- layer-2 attention, same two i-pieces as layer 1 ----
    fin = persist.tile([D, SLICE], F32, tag="fin", name="fin")
    a_groups = [[q * 4 + tl for tl in range(3)] for q in range(8)]
    b_singles = [q * 4 + 3 for q in range(8)]
    for pi, (off, pw) in enumerate(PIECES):
        acc2 = psum.tile([D + 1, pw], F32, tag="acc", bufs=2,
                         name=f"acc2_{pi}")
        n_done = 0
        for grp in a_groups + [[t] for t in b_singles]:
            gl = len(grp)
            vg = work.tile([P, gl * pw], F16, tag="vg", bufs=3,
                           name=f"vg2_{pi}_{grp[0]}")
            for k, t in enumerate(grp):
                nc.vector.tensor_scalar(
                    vg[:, k * pw:(k + 1) * pw], g2_row[:, off:off + pw],
                    rp_2[:, t:t + 1], e1_2[:, t:t + 1], ALU.mult, ALU.max)
            wg = work.tile([P, gl * pw], F16, tag="wg", bufs=4,
                           name=f"wg2_{pi}_{grp[0]}")
            nc.vector.tensor_tensor(wg[:], vg[:],
                                    adj_piece(pi, grp[0], gl), ALU.mult)
            for k, t in enumerate(grp):
                n_done += 1
                nc.tensor.matmul(
                    acc2[:], whs2[:, t * (D + 1):(t + 1) * (D + 1)],
                    wg[:, k * pw:(k + 1) * pw],
                    start=(t == 0), stop=(n_done == NT))
        num2 = work.tile([D + 1, pw], F32, tag="num", bufs=3,
                         name=f"num_l2_{pi}")
        nc.scalar.activation(num2[:], acc2[:], AF.Copy)
        lnr2 = work.tile([1, pw], F32, tag="lnr", bufs=2, name=f"ln_l2_{pi}")
        nc.scalar.activation(lnr2[:], num2[D:D + 1, :], AF.Ln)
        rec2 = work.tile([1, pw], F32, tag="rec", bufs=2, name=f"rc_l2_{pi}")
        nc.scalar.activation(rec2[:], lnr2[:], AF.Exp, scale=-1.0)
        den2 = psum.tile([D, pw], F32, tag="bank", bufs=3,
                         name=f"den_l2_{pi}")
        nc.tensor.matmul(den2[:], ones_all[0:1, 0:D], rec2[0:1, :],
                         start=True, stop=True)
        x2 = work.tile([D, pw], F32, tag="xat", bufs=3, name=f"x_l2_{pi}")
        nc.vector.tensor_tensor(x2[:], num2[0:D, :], den2[:], ALU.mult)
        elu(f"l2_{pi}", x2[:], fin[:, off:off + pw], pw)
    nc.sync.dma_start(out=outT[:], in_=fin[:])


# ---------------------------------------------------------------------------
# host-side driver
# ---------------------------------------------------------------------------

def _prep_inputs(x, adj, W, a, Wo, ao):
    xT = x.T.astype(np.float16)                       # [F, N]
    xtr = np.ascontiguousarray(
        xT.reshape(NKF, P, N).transpose(1, 0, 2).reshape(P, NKF * N))
    wext = np.empty((F, H, E), np.float32)
    for h in range(H):
        a_src, a_dst = a[h, :D], a[h, D:]
        wext[:, h, 0:D] = W[h]
        wext[:, h, D] = W[h] @ a_src
        wext[:, h, D + 1] = W[h] @ a_dst
    wextr = np.ascontiguousarray(
        wext.reshape(NKF, P, H * E).transpose(1, 0, 2).reshape(P, -1)
    ).astype(np.float16)
    woAm = np.empty((D, H, E), np.float32)
    for h in range(H):
        Wo_h = Wo[h * D:(h + 1) * D]                  # [64 feat, 64 class]
        woAm[:, h, 0:D] = Wo_h
        woAm[:, h, D] = Wo_h @ ao[:D]
        woAm[:, h, D + 1] = Wo_h @ ao[D:]
    woAr = np.ascontiguousarray(woAm.reshape(D, H * E)).astype(np.float16)

    adjT = adj.T.astype(np.float16)                   # [j, i]
    in_maps = []
    for c in range(N_CORES):
        sl = slice(c * SLICE, (c + 1) * SLICE)
        asl = adjT[:, sl]
        blocks = []
        for off, pw in PIECES:
            blocks.append(
                asl[:, off:off + pw].reshape(NT, P, pw).transpose(1, 0, 2)
                .reshape(P, NT * pw))
        adjcm = np.ascontiguousarray(np.concatenate(blocks, axis=1))
        xslr = np.ascontiguousarray(
            xT[:, sl].reshape(NKF, P, SLICE).transpose(1, 0, 2)
            .reshape(P, NKF * SLICE))
        in_maps.append({
            "xtr": xtr, "xslr": xslr, "adjc": adjcm,
            "wextr": wextr, "woA": woAr,
        })
    return in_maps


def kernel(x, adj, W, a, Wo, ao, cfg):
    x = np.asarray(x, np.float32)
    adj = np.asarray(adj, np.float32)
    W = np.asarray(W, np.float32)
    a = np.asarray(a, np.float32)
    Wo = np.asarray(Wo, np.float32)
    ao = np.asarray(ao, np.float32)

    in_maps = _prep_inputs(x, adj, W, a, Wo, ao)
    if _CACHED.get("nc") is None:
        _CACHED["nc"] = build_kernel()
    res = run_bass_kernel_spmd(_CACHED["nc"], in_maps,
                               core_ids=list(range(N_CORES)))
    out = np.empty((N, D), np.float32)
    for c in range(N_CORES):
        out[c * SLICE:(c + 1) * SLICE, :] = res.results[c]["outT"].T
    return out


if __name__ == "__main__":
    import reference as ref_mod
    inputs = {k: np.asarray(v) for k, v in ref_mod.setup_inputs().items()}
    expected = np.asarray(ref_mod.reference(**ref_mod.setup_inputs()))
    got = kernel(**inputs)
    err = np.abs(got - expected).max() / np.abs(expected).max()
    print("rel err:", err)
